# revision 12
# baseline (speedup 1.0000x reference)
"""Trainium2 Bass kernel for nn_DUDCLoss_1382979469646.

Data-parallel over the batch dim: 8 cores x 512 rows each. The loss is
factorized so each row needs only a handful of C-length passes; the
device computes five per-row reductions and the host finishes the tiny
O(B*K) assembly in fp64 (mirroring the host-side gather the input
already requires).

Single (masked-softmax) part: with A=exp(x), E=sum(A), a_k=A[pos_k],
En=E-sum_k(a_k), D_j=En+a_j, the per-(row,j) cross-entropy is
  xent12_j = ln(D2_j) - (G12 - S12 + a1_j*g2_j) / D1_j
where G12 = sum_c A1_c * x2_c and S12 = sum_k a1_k * g2_k. This uses
ln(p+eps) ~= ln(p) (drops the +eps inside the log); measured total error
~1.2e-3 relative - well inside the 2e-2 gate. Dropping eps makes the
log of the softmax numerator the input logit itself, so no ln pass is
needed for the cross terms. Device ships E1,E2,G12,G21 per row; host
(which already gathered g = x[pos]) forms D, ln(D), 1/D over [B,K].

Multi (sigmoid) part: ln(sigmoid(x)+eps) ~= u = x - ln(1+A), and
sigmoid(x) = exp(u) computed as pow(e, u) on the gpsimd engine - so the
ACT engine runs only two big passes (Exp, Ln[1+A]), both in the one ACT
table set that holds them (a patched table-selection policy guarantees
a single ~1.3us table load). Device ships sum(s1*u2 + s2*u1) per tile.

Scheduling: the tile scheduler orders each engine's queue by readiness,
which lets exp(t+1) preempt ln1p(t) and starves the downstream
gpsimd/DVE pipeline - so exp(t+1) carries a zero bias AP produced (on
the prompt gpsimd queue) from ln1p(t)'s output, forcing the
exp/ln1p/exp... alternation. The last tile runs in quarter-columns:
u-subtract quarters first, sigmoid quarters split between ACT exp(u)
(idle by then) and gpsimd pow, each followed immediately by its product
and accumulator so only ~1us trails the engine streams.

Engine budget per core (cost model): ACT ~16us, DVE ~18us (row
accumulators via tensor_scalar accum in 4x perf mode, products via
tensor_tensor in 2x - all operands 2-byte), Pool ~15us, DMA ~6us (fp16
inputs, converted on host).
"""

import numpy as np

NCORES = 8
B, C, K = 4096, 1024, 8
RPC = B // NCORES          # rows per core
P = 128                    # partitions
T = RPC // P               # row-tiles per core
TK = T * K
EPS = 1e-5
OUTW = 4 * T + (T - 1) + 4  # E1,E2,G12,G21 | M per tile | M quarter-splits

_cache = {}


def _patch_act_tables(mybir, bacc):
    """Make the ACT-table-load inserter resolve both Exp and Ln to the one
    set that holds both (natural_log_exp_and_others). The default policy
    picks a singleton set per function, inserting a ~1.3us table load at
    every Exp<->Ln transition in the scheduled stream."""
    if getattr(bacc, "_dudc_act_patch", False):
        return
    orig = bacc.get_activation_tables
    both = {mybir.ActivationFunctionType.Exp, mybir.ActivationFunctionType.Ln}

    def patched(arch):
        tabs = orig(arch)
        if any(both <= funcs for funcs in tabs.values()):
            for name, funcs in tabs.items():
                if not both <= funcs:
                    funcs.difference_update(both)
        return tabs

    bacc.get_activation_tables = patched
    bacc._dudc_act_patch = True


def _build():
    import concourse.bass as bass
    import concourse.tile as tile
    from concourse import bacc, mybir

    _patch_act_tables(mybir, bacc)

    fp32 = mybir.dt.float32
    fp16 = mybir.dt.float16
    AF = mybir.ActivationFunctionType
    ALU = mybir.AluOpType

    nc = bacc.Bacc(
        "TRN2",
        target_bir_lowering=False,
        debug=False,
        num_devices=NCORES,
    )

    x1d = nc.dram_tensor("x1", [RPC, C], fp16, kind="ExternalInput").ap()
    x2d = nc.dram_tensor("x2", [RPC, C], fp16, kind="ExternalInput").ap()
    outd = nc.dram_tensor("out", [P, OUTW], fp32, kind="ExternalOutput").ap()

    cE1, cE2, cG12, cG21, cM = 0, T, 2 * T, 3 * T, 4 * T

    with tile.TileContext(nc) as tc:
        with (
            tc.tile_pool(name="x", bufs=4) as xp,
            tc.tile_pool(name="A", bufs=3) as ap_,
            tc.tile_pool(name="llp", bufs=3) as llpp,
            tc.tile_pool(name="u", bufs=3) as up,
            tc.tile_pool(name="sg", bufs=3) as sgp,
            tc.tile_pool(name="q", bufs=4) as qp,
            tc.tile_pool(name="scratch", bufs=6) as scp,
            tc.tile_pool(name="small", bufs=1) as sm,
        ):
            outt = sm.tile([P, OUTW], fp32)

            # base-e constant for pow(e, u) = exp(u) on gpsimd; memset runs
            # on the otherwise-idle DVE during the first input DMA
            et = sm.tile([P, 2 * C], fp16)
            nc.vector.memset(et[:], float(np.e))

            # primer: a no-dependency ACT instruction so the ~1.3us ACT table
            # load (inserted before the first activation in the scheduled
            # stream) runs at t=0 instead of behind the first input DMA
            dm = sm.tile([P, 1], fp32)
            dmo = sm.tile([P, 1], fp32)
            nc.vector.memset(dm[:], 0.0)
            nc.scalar.activation(dmo[:], dm[:], AF.Exp)

            def acc(dst_col, src_ap):
                # fast row accumulate: tensor_scalar keeps 4x perf mode
                scw = scp.tile([P, 2 * C], fp16, tag="scw")
                w = src_ap.shape[-1]
                nc.vector.tensor_scalar(
                    scw[:, 0:w], src_ap, 1.0, 0.0, op0=ALU.mult, op1=ALU.add,
                    accum_out=outt[:, dst_col : dst_col + 1],
                )

            zbs = []   # [P,1] zero bias tiles forcing exp(t+1) after ln1p(t)
            uts = []
            for t in range(T):
                r0, r1 = t * P, (t + 1) * P
                if t == 0:
                    # tile 0 on two separate tiles: per-tensor deps so exp of
                    # the x2 half starts as soon as its own DMA lands
                    xta = xp.tile([P, C], fp16, tag="xa")
                    xtb = xp.tile([P, C], fp16, tag="xb")
                    nc.gpsimd.dma_start(xtb[:], x2d[r0:r1, :])
                    nc.gpsimd.dma_start(xta[:], x1d[r0:r1, :])
                    xparts = [(xtb, slice(C, 2 * C)), (xta, slice(0, C))]
                    x1v, x2v = xta[:], xtb[:]
                else:
                    xt = xp.tile([P, 2 * C], fp16, tag="x")
                    nc.sync.dma_start(xt[:, 0:C], x1d[r0:r1, :])
                    nc.sync.dma_start(xt[:, C : 2 * C], x2d[r0:r1, :])
                    xparts = [(xt, slice(0, 2 * C))]
                    x1v, x2v = xt[:, 0:C], xt[:, C : 2 * C]

                At = ap_.tile([P, 2 * C], fp16, tag="A")
                if t == 0:
                    for xsrc, dsl in xparts:
                        nc.scalar.activation(At[:, dsl], xsrc[:], AF.Exp)
                else:
                    # zero bias from ln1p(t-1) pins queue order exp/ln1p/...
                    nc.scalar.activation(At[:], xt[:], AF.Exp, bias=zbs[t - 1][:])

                # row sums E1, E2 straight into out columns (x2 half of At
                # lands first on tile 0, so accumulate it first)
                if t == 0:
                    acc(cE2 + t, At[:, C : 2 * C])
                    acc(cE1 + t, At[:, 0:C])
                else:
                    acc(cE1 + t, At[:, 0:C])
                    acc(cE2 + t, At[:, C : 2 * C])

                # cross products G12 = sum A1*x2, G21 = sum A2*x1; tile 0's
                # multiplies fill the gpsimd queue's initial idle window
                qg = qp.tile([P, 2 * C], fp16, tag="qg")
                eng = nc.gpsimd if t == 0 else nc.vector
                if t == 0:
                    eng.tensor_tensor(
                        qg[:, C : 2 * C], At[:, C : 2 * C], x1v, op=ALU.mult
                    )
                    eng.tensor_tensor(qg[:, 0:C], At[:, 0:C], x2v, op=ALU.mult)
                    acc(cG21 + t, qg[:, C : 2 * C])
                    acc(cG12 + t, qg[:, 0:C])
                else:
                    eng.tensor_tensor(qg[:, 0:C], At[:, 0:C], x2v, op=ALU.mult)
                    eng.tensor_tensor(
                        qg[:, C : 2 * C], At[:, C : 2 * C], x1v, op=ALU.mult
                    )
                    acc(cG12 + t, qg[:, 0:C])
                    acc(cG21 + t, qg[:, C : 2 * C])

                # u = x - ln(1+A) = log(sigmoid(x)); subtract and
                # s = exp(u) = pow(e, u) on the gpsimd engine
                LLpt = llpp.tile([P, 2 * C], fp32, tag="llp")
                if t == T - 1:
                    nc.scalar.activation(LLpt[:, 0:C], At[:, 0:C], AF.Ln, bias=1.0)
                    nc.scalar.activation(
                        LLpt[:, C : 2 * C], At[:, C : 2 * C], AF.Ln, bias=1.0
                    )
                else:
                    nc.scalar.activation(LLpt[:], At[:], AF.Ln, bias=1.0)
                if t < T - 1:
                    # zero bias via a tiny same-engine Copy: orders the ACT
                    # queue exp/ln1p/exp/... with no cross-engine semaphore
                    zb = sm.tile([P, 1], fp32)
                    nc.scalar.activation(zb[:], LLpt[:, 0:1], AF.Copy, scale=0.0)
                    zbs.append(zb)
                ut = up.tile([P, 2 * C], fp16, tag="u")
                sgt = sgp.tile([P, 2 * C], fp16, tag="sg")
                if t == 0:
                    nc.gpsimd.tensor_sub(ut[:, 0:C], xta[:], LLpt[:, 0:C])
                    nc.gpsimd.tensor_sub(ut[:, C : 2 * C], xtb[:], LLpt[:, C : 2 * C])
                    nc.gpsimd.tensor_tensor(sgt[:], et[:], ut[:], op=ALU.pow)
                elif t < T - 1:
                    nc.gpsimd.tensor_sub(ut[:], xt[:], LLpt[:])
                    nc.gpsimd.tensor_tensor(sgt[:], et[:], ut[:], op=ALU.pow)
                uts.append((ut, sgt))

                # multi products: qm = s * u_swapped; M12+M21 into one
                # accumulator per tile (only the total enters the loss)
                if t < T - 1:
                    qm = qp.tile([P, 2 * C], fp16, tag="qm")
                    nc.vector.tensor_mul(qm[:, 0:C], sgt[:, 0:C], ut[:, C : 2 * C])
                    nc.vector.tensor_mul(
                        qm[:, C : 2 * C], sgt[:, C : 2 * C], ut[:, 0:C]
                    )
                    acc(cM + t, qm[:])
                else:
                    # last tile in quarter-columns: u quarters interleaved
                    # with the q1/q2 sigmoids on gpsimd; q3/q4 sigmoids on the
                    # now-idle ACT; products + accumulators on DVE as soon as
                    # each (sigmoid, partner-u) pair is ready
                    Q = (2 * C) // 4
                    qs = [slice(i * Q, (i + 1) * Q) for i in range(4)]
                    def usub_q(i):
                        nc.gpsimd.tensor_sub(
                            ut[:, qs[i]], xt[:, qs[i]], LLpt[:, qs[i]]
                        )
                    # u quarters ordered so the ACT-bound sigmoid inputs (q2,
                    # q3) land first; pow quarters (q0, q1) follow on gpsimd
                    usub_q(2)
                    usub_q(0)
                    nc.scalar.activation(sgt[:, qs[2]], ut[:, qs[2]], AF.Exp)
                    usub_q(3)
                    usub_q(1)
                    nc.scalar.activation(sgt[:, qs[3]], ut[:, qs[3]], AF.Exp)
                    nc.gpsimd.tensor_tensor(
                        sgt[:, qs[0]], et[:, qs[0]], ut[:, qs[0]], op=ALU.pow
                    )
                    nc.gpsimd.tensor_tensor(
                        sgt[:, qs[1]], et[:, qs[1]], ut[:, qs[1]], op=ALU.pow
                    )
                    # products alternate gpsimd/DVE so the accumulators
                    # interleave instead of trailing a serial product chain
                    for n, i in enumerate((2, 0, 3, 1)):
                        j = (i + 2) % 4
                        qm = qp.tile([P, Q], fp16, tag=f"qmq{i}")
                        if i in (2, 3):
                            nc.vector.tensor_mul(qm[:], sgt[:, qs[i]], ut[:, qs[j]])
                        else:
                            nc.gpsimd.tensor_tensor(
                                qm[:], sgt[:, qs[i]], ut[:, qs[j]], op=ALU.mult
                            )
                        acc(cM + t + n, qm[:])

            nc.gpsimd.dma_start(outd, outt[:])

    nc.compile()
    return nc


def _get_nc():
    if "nc" not in _cache:
        _cache["nc"] = _build()
    return _cache["nc"]


def kernel(out1, out2, para, target, pos_idx):
    from concourse.bass_utils import run_bass_kernel_spmd

    nc = _get_nc()

    out1 = np.ascontiguousarray(out1, dtype=np.float32)
    out2 = np.ascontiguousarray(out2, dtype=np.float32)
    x1h = out1.astype(np.float16)
    x2h = out2.astype(np.float16)

    in_maps = [
        {
            "x1": x1h[c * RPC : (c + 1) * RPC],
            "x2": x2h[c * RPC : (c + 1) * RPC],
        }
        for c in range(NCORES)
    ]
    res = run_bass_kernel_spmd(nc, in_maps, core_ids=list(range(NCORES)))
    parts = np.stack([r["out"] for r in res.results])  # [NCORES, P, OUTW]

    def rows(col0):
        # device cols [col0 : col0+T], laid out [core, p, t] -> row c*RPC+t*P+p
        return (
            parts[:, :, col0 : col0 + T]
            .transpose(0, 2, 1)
            .reshape(B)
            .astype(np.float64)
        )

    E1, E2 = rows(0), rows(T)
    G12, G21 = rows(2 * T), rows(3 * T)
    Msum = parts[:, :, 4 * T :].sum(dtype=np.float64)

    # host assembly over [B, K] in fp64 (g already gathered on host anyway)
    idx = pos_idx.astype(np.int64)
    g1 = np.take_along_axis(out1, idx, axis=1).astype(np.float64)
    g2 = np.take_along_axis(out2, idx, axis=1).astype(np.float64)
    a1, a2 = np.exp(g1), np.exp(g2)
    D1 = (E1 - a1.sum(1))[:, None] + a1
    D2 = (E2 - a2.sum(1))[:, None] + a2
    r1, r2 = 1.0 / D1, 1.0 / D2
    row_single = (
        np.log(D1).sum(1) + np.log(D2).sum(1)
        - (G12 - (a1 * g2).sum(1)) * r1.sum(1)
        - (G21 - (a2 * g1).sum(1)) * r2.sum(1)
        - (a1 * g2 * r1).sum(1)
        - (a2 * g1 * r2).sum(1)
    )
    single = row_single.sum() / (B * K)
    multi = -Msum / B
    p = float(np.asarray(para))
    return np.asarray(p * multi + (1.0 - p) * single, dtype=np.float32)


# revision 13
# speedup vs baseline: 1.0255x; 1.0255x over previous
"""Trainium2 Bass kernel for nn_DUDCLoss_1382979469646.

Data-parallel over the batch dim: 8 cores x 512 rows each. The loss is
factorized so each row needs only a handful of C-length passes; the
device computes five per-row reductions and the host finishes the tiny
O(B*K) assembly in fp64 (mirroring the host-side gather the input
already requires).

Single (masked-softmax) part: with A=exp(x), E=sum(A), a_k=A[pos_k],
En=E-sum_k(a_k), D_j=En+a_j, the per-(row,j) cross-entropy is
  xent12_j = ln(D2_j) - (G12 - S12 + a1_j*g2_j) / D1_j
where G12 = sum_c A1_c * x2_c and S12 = sum_k a1_k * g2_k. This uses
ln(p+eps) ~= ln(p) (drops the +eps inside the log); measured total error
~1.2e-3 relative - well inside the 2e-2 gate. Dropping eps makes the
log of the softmax numerator the input logit itself, so no ln pass is
needed for the cross terms. Device ships E1,E2,G12,G21 per row; host
(which already gathered g = x[pos]) forms D, ln(D), 1/D over [B,K].

Multi (sigmoid) part: ln(sigmoid(x)+eps) ~= u = x - ln(1+A), and
sigmoid(x) = exp(u) computed as pow(e, u) on the gpsimd engine - so the
ACT engine runs only two big passes (Exp, Ln[1+A]), both in the one ACT
table set that holds them (a patched table-selection policy guarantees
a single ~1.3us table load). Device ships sum(s1*u2 + s2*u1) per tile.

Scheduling: the tile scheduler orders each engine's queue by readiness,
which lets exp(t+1) preempt ln1p(t) and starves the downstream
gpsimd/DVE pipeline - so exp(t+1) carries a zero bias AP produced (on
the prompt gpsimd queue) from ln1p(t)'s output, forcing the
exp/ln1p/exp... alternation. The last tile runs in quarter-columns:
u-subtract quarters first, sigmoid quarters split between ACT exp(u)
(idle by then) and gpsimd pow, each followed immediately by its product
and accumulator so only ~1us trails the engine streams.

Engine budget per core (cost model): ACT ~16us, DVE ~18us (row
accumulators via tensor_scalar accum in 4x perf mode, products via
tensor_tensor in 2x - all operands 2-byte), Pool ~15us, DMA ~6us (fp16
inputs, converted on host).
"""

import numpy as np

NCORES = 8
B, C, K = 4096, 1024, 8
RPC = B // NCORES          # rows per core
P = 128                    # partitions
T = RPC // P               # row-tiles per core
TK = T * K
EPS = 1e-5
OUTW = 4 * T + (T - 1) + 4  # E1,E2,G12,G21 | M per tile | M quarter-splits

_cache = {}


def _patch_act_tables(mybir, bacc):
    """Make the ACT-table-load inserter resolve both Exp and Ln to the one
    set that holds both (natural_log_exp_and_others). The default policy
    picks a singleton set per function, inserting a ~1.3us table load at
    every Exp<->Ln transition in the scheduled stream."""
    if getattr(bacc, "_dudc_act_patch", False):
        return
    orig = bacc.get_activation_tables
    both = {mybir.ActivationFunctionType.Exp, mybir.ActivationFunctionType.Ln}

    def patched(arch):
        tabs = orig(arch)
        if any(both <= funcs for funcs in tabs.values()):
            for name, funcs in tabs.items():
                if not both <= funcs:
                    funcs.difference_update(both)
        return tabs

    bacc.get_activation_tables = patched
    bacc._dudc_act_patch = True


def _build():
    import concourse.bass as bass
    import concourse.tile as tile
    from concourse import bacc, mybir

    _patch_act_tables(mybir, bacc)

    fp32 = mybir.dt.float32
    fp16 = mybir.dt.float16
    AF = mybir.ActivationFunctionType
    ALU = mybir.AluOpType

    nc = bacc.Bacc(
        "TRN2",
        target_bir_lowering=False,
        debug=False,
        num_devices=NCORES,
    )

    x1d = nc.dram_tensor("x1", [RPC, C], fp16, kind="ExternalInput").ap()
    x2d = nc.dram_tensor("x2", [RPC, C], fp16, kind="ExternalInput").ap()
    outd = nc.dram_tensor("out", [P, OUTW], fp32, kind="ExternalOutput").ap()

    cE1, cE2, cG12, cG21, cM = 0, T, 2 * T, 3 * T, 4 * T

    with tile.TileContext(nc) as tc:
        with (
            tc.tile_pool(name="x", bufs=4) as xp,
            tc.tile_pool(name="A", bufs=3) as ap_,
            tc.tile_pool(name="llp", bufs=3) as llpp,
            tc.tile_pool(name="u", bufs=3) as up,
            tc.tile_pool(name="sg", bufs=3) as sgp,
            tc.tile_pool(name="q", bufs=4) as qp,
            tc.tile_pool(name="scratch", bufs=6) as scp,
            tc.tile_pool(name="small", bufs=1) as sm,
        ):
            outt = sm.tile([P, OUTW], fp32)

            # base-e constant for pow(e, u) = exp(u) on gpsimd; memset runs
            # on the otherwise-idle DVE during the first input DMA
            et = sm.tile([P, 2 * C], fp16)
            nc.vector.memset(et[:], float(np.e))

            # primer: a no-dependency ACT instruction so the ~1.3us ACT table
            # load (inserted before the first activation in the scheduled
            # stream) runs at t=0 instead of behind the first input DMA
            dm = sm.tile([P, 1], fp32)
            dmo = sm.tile([P, 1], fp32)
            nc.vector.memset(dm[:], 0.0)
            nc.scalar.activation(dmo[:], dm[:], AF.Exp)

            def acc(dst_col, src_ap):
                # fast row accumulate: tensor_scalar keeps 4x perf mode
                scw = scp.tile([P, 2 * C], fp16, tag="scw")
                w = src_ap.shape[-1]
                nc.vector.tensor_scalar(
                    scw[:, 0:w], src_ap, 1.0, 0.0, op0=ALU.mult, op1=ALU.add,
                    accum_out=outt[:, dst_col : dst_col + 1],
                )

            zbs = []   # [P,1] zero bias tiles forcing exp(t+1) after ln1p(t)
            uts = []
            for t in range(T):
                r0, r1 = t * P, (t + 1) * P
                if t == 0:
                    # tile 0 on two separate tiles: per-tensor deps so exp of
                    # the x2 half starts as soon as its own DMA lands
                    xta = xp.tile([P, C], fp16, tag="xa")
                    xtb = xp.tile([P, C], fp16, tag="xb")
                    nc.gpsimd.dma_start(xtb[:], x2d[r0:r1, :])
                    nc.gpsimd.dma_start(xta[:], x1d[r0:r1, :])
                    xparts = [(xtb, slice(C, 2 * C)), (xta, slice(0, C))]
                    x1v, x2v = xta[:], xtb[:]
                else:
                    xt = xp.tile([P, 2 * C], fp16, tag="x")
                    nc.sync.dma_start(xt[:, 0:C], x1d[r0:r1, :])
                    nc.sync.dma_start(xt[:, C : 2 * C], x2d[r0:r1, :])
                    xparts = [(xt, slice(0, 2 * C))]
                    x1v, x2v = xt[:, 0:C], xt[:, C : 2 * C]

                At = ap_.tile([P, 2 * C], fp16, tag="A")
                if t == 0:
                    for xsrc, dsl in xparts:
                        nc.scalar.activation(At[:, dsl], xsrc[:], AF.Exp)
                else:
                    # zero bias from ln1p(t-1) pins queue order exp/ln1p/...
                    nc.scalar.activation(At[:], xt[:], AF.Exp, bias=zbs[t - 1][:])

                # row sums E1, E2 straight into out columns (x2 half of At
                # lands first on tile 0, so accumulate it first)
                if t == 0:
                    acc(cE2 + t, At[:, C : 2 * C])
                    acc(cE1 + t, At[:, 0:C])
                else:
                    acc(cE1 + t, At[:, 0:C])
                    acc(cE2 + t, At[:, C : 2 * C])

                # cross products G12 = sum A1*x2, G21 = sum A2*x1; tile 0's
                # multiplies fill the gpsimd queue's initial idle window
                qg = qp.tile([P, 2 * C], fp16, tag="qg")
                eng = nc.gpsimd if t == 0 else nc.vector
                if t == 0:
                    eng.tensor_tensor(
                        qg[:, C : 2 * C], At[:, C : 2 * C], x1v, op=ALU.mult
                    )
                    eng.tensor_tensor(qg[:, 0:C], At[:, 0:C], x2v, op=ALU.mult)
                    acc(cG21 + t, qg[:, C : 2 * C])
                    acc(cG12 + t, qg[:, 0:C])
                else:
                    eng.tensor_tensor(qg[:, 0:C], At[:, 0:C], x2v, op=ALU.mult)
                    eng.tensor_tensor(
                        qg[:, C : 2 * C], At[:, C : 2 * C], x1v, op=ALU.mult
                    )
                    acc(cG12 + t, qg[:, 0:C])
                    acc(cG21 + t, qg[:, C : 2 * C])

                # u = x - ln(1+A) = log(sigmoid(x)); subtract and
                # s = exp(u) = pow(e, u) on the gpsimd engine
                LLpt = llpp.tile([P, 2 * C], fp32, tag="llp")
                if t == T - 1:
                    nc.scalar.activation(LLpt[:, 0:C], At[:, 0:C], AF.Ln, bias=1.0)
                    nc.scalar.activation(
                        LLpt[:, C : 2 * C], At[:, C : 2 * C], AF.Ln, bias=1.0
                    )
                else:
                    nc.scalar.activation(LLpt[:], At[:], AF.Ln, bias=1.0)
                if t < T - 1:
                    # zero bias via a tiny same-engine Copy: orders the ACT
                    # queue exp/ln1p/exp/... with no cross-engine semaphore
                    zb = sm.tile([P, 1], fp32)
                    nc.scalar.activation(zb[:], LLpt[:, 0:1], AF.Copy, scale=0.0)
                    zbs.append(zb)
                ut = up.tile([P, 2 * C], fp16, tag="u")
                sgt = sgp.tile([P, 2 * C], fp16, tag="sg")
                if t == 0:
                    nc.gpsimd.tensor_sub(ut[:, 0:C], xta[:], LLpt[:, 0:C])
                    nc.gpsimd.tensor_sub(ut[:, C : 2 * C], xtb[:], LLpt[:, C : 2 * C])
                    nc.gpsimd.tensor_tensor(sgt[:], et[:], ut[:], op=ALU.pow)
                elif t < T - 1:
                    nc.gpsimd.tensor_sub(ut[:], xt[:], LLpt[:])
                    nc.gpsimd.tensor_tensor(sgt[:], et[:], ut[:], op=ALU.pow)
                uts.append((ut, sgt))

                # multi products: qm = s * u_swapped; M12+M21 into one
                # accumulator per tile (only the total enters the loss)
                if t < T - 1:
                    qm = qp.tile([P, 2 * C], fp16, tag="qm")
                    nc.vector.tensor_mul(qm[:, 0:C], sgt[:, 0:C], ut[:, C : 2 * C])
                    nc.vector.tensor_mul(
                        qm[:, C : 2 * C], sgt[:, C : 2 * C], ut[:, 0:C]
                    )
                    acc(cM + t, qm[:])
                else:
                    # last tile in quarter-columns: u quarters interleaved
                    # with the q1/q2 sigmoids on gpsimd; q3/q4 sigmoids on the
                    # now-idle ACT; products + accumulators on DVE as soon as
                    # each (sigmoid, partner-u) pair is ready
                    Q = (2 * C) // 4
                    qs = [slice(i * Q, (i + 1) * Q) for i in range(4)]
                    def usub_q(i):
                        nc.gpsimd.tensor_sub(
                            ut[:, qs[i]], xt[:, qs[i]], LLpt[:, qs[i]]
                        )
                    # u quarters ordered so the ACT-bound sigmoid inputs (q2,
                    # q3) land first; pow quarters (q0, q1) follow on gpsimd
                    usub_q(2)
                    usub_q(0)
                    nc.scalar.activation(sgt[:, qs[2]], ut[:, qs[2]], AF.Exp)
                    usub_q(3)
                    usub_q(1)
                    nc.scalar.activation(sgt[:, qs[3]], ut[:, qs[3]], AF.Exp)
                    nc.gpsimd.tensor_tensor(
                        sgt[:, qs[0]], et[:, qs[0]], ut[:, qs[0]], op=ALU.pow
                    )
                    nc.gpsimd.tensor_tensor(
                        sgt[:, qs[1]], et[:, qs[1]], ut[:, qs[1]], op=ALU.pow
                    )
                    # products on gpsimd (free after the pow quarters), only
                    # the accumulators on DVE
                    for n, i in enumerate((2, 0, 3, 1)):
                        j = (i + 2) % 4
                        qm = qp.tile([P, Q], fp16, tag=f"qmq{i}")
                        nc.gpsimd.tensor_tensor(
                            qm[:], sgt[:, qs[i]], ut[:, qs[j]], op=ALU.mult
                        )
                        acc(cM + t + n, qm[:])

            nc.gpsimd.dma_start(outd, outt[:])

    nc.compile()
    return nc


def _get_nc():
    if "nc" not in _cache:
        _cache["nc"] = _build()
    return _cache["nc"]


def kernel(out1, out2, para, target, pos_idx):
    from concourse.bass_utils import run_bass_kernel_spmd

    nc = _get_nc()

    out1 = np.ascontiguousarray(out1, dtype=np.float32)
    out2 = np.ascontiguousarray(out2, dtype=np.float32)
    x1h = out1.astype(np.float16)
    x2h = out2.astype(np.float16)

    in_maps = [
        {
            "x1": x1h[c * RPC : (c + 1) * RPC],
            "x2": x2h[c * RPC : (c + 1) * RPC],
        }
        for c in range(NCORES)
    ]
    res = run_bass_kernel_spmd(nc, in_maps, core_ids=list(range(NCORES)))
    parts = np.stack([r["out"] for r in res.results])  # [NCORES, P, OUTW]

    def rows(col0):
        # device cols [col0 : col0+T], laid out [core, p, t] -> row c*RPC+t*P+p
        return (
            parts[:, :, col0 : col0 + T]
            .transpose(0, 2, 1)
            .reshape(B)
            .astype(np.float64)
        )

    E1, E2 = rows(0), rows(T)
    G12, G21 = rows(2 * T), rows(3 * T)
    Msum = parts[:, :, 4 * T :].sum(dtype=np.float64)

    # host assembly over [B, K] in fp64 (g already gathered on host anyway)
    idx = pos_idx.astype(np.int64)
    g1 = np.take_along_axis(out1, idx, axis=1).astype(np.float64)
    g2 = np.take_along_axis(out2, idx, axis=1).astype(np.float64)
    a1, a2 = np.exp(g1), np.exp(g2)
    D1 = (E1 - a1.sum(1))[:, None] + a1
    D2 = (E2 - a2.sum(1))[:, None] + a2
    r1, r2 = 1.0 / D1, 1.0 / D2
    row_single = (
        np.log(D1).sum(1) + np.log(D2).sum(1)
        - (G12 - (a1 * g2).sum(1)) * r1.sum(1)
        - (G21 - (a2 * g1).sum(1)) * r2.sum(1)
        - (a1 * g2 * r1).sum(1)
        - (a2 * g1 * r2).sum(1)
    )
    single = row_single.sum() / (B * K)
    multi = -Msum / B
    p = float(np.asarray(para))
    return np.asarray(p * multi + (1.0 - p) * single, dtype=np.float32)
